# revision 27
# baseline (speedup 1.0000x reference)
"""BaseGIN (3-layer GIN + MLP + BN + residual) Trainium2 Bass kernel, 8-core SPMD.

v2 architecture ("edge-slab / PE-aggregation"):
- Nodes sharded 8 ways (6250/core) and degree-sorted per core (boustrophedon
  by lo-degree, snake on hi).  Tables (x and per-layer outputs) live in DRAM
  in sorted-concatenated order [core0-sorted | ... | core7-sorted], bf16, so
  one gather-index table serves all 3 layers and no per-layer unpermute is
  needed.  Sources split into two int16-addressable regions at the owner-core
  boundary (cores 0-4 = rows [0,31250), cores 5-7 = rest).
- Edges are packed edge-major into 128-slot slabs; each slab covers a
  contiguous window of <=WMAX destination nodes (window boundaries computed
  on the cross-core max degree profile, so the program is SPMD-uniform).
- Aggregation is done on the PE: per slab, matmul(lhsT=gathered-messages
  [128e x 128f], rhs=A [128e x W]) accumulates the weighted segment sum
  feature-major into PSUM.  A = onehot(localnode) * edge_weight is built
  on DVE from two tiny tables (2 ops per chunk side).  The (1+eps)*x term
  initializes PSUM via a scaled-identity transpose matmul.
- MLP runs feature-major on the PSUM result (bf16 GEMMs); BN batch stats via
  accum_out + a 1KB AllReduce; BN+ReLU fused in one ACT op; transpose back
  and residual-add into the persistent bf16 node-major shard.
- Layer boundary: shard -> local DRAM -> CCE AllGather (bf16) -> next table.
"""
import sys

sys.path.insert(0, "/opt/trn_rl_repo")
import numpy as np
import ml_dtypes
import concourse.bass as bass
import concourse.bacc as bacc
import concourse.mybir as mybir
import concourse.tile as tile
from concourse import library_config
from concourse.bass_utils import run_bass_kernel_spmd

F32 = mybir.dt.float32
BF16 = mybir.dt.bfloat16
I16 = mybir.dt.int16
Alu = mybir.AluOpType
Act = mybir.ActivationFunctionType
Ax = mybir.AxisListType
BFNP = ml_dtypes.bfloat16

D = 128
C = 8
BN_EPS = 1e-5
NN = 50000
NLOC = NN // C
NPAD = 6272
NB = NPAD // 128
CH = 512
RA = 5 * NLOC          # region A = table rows [0, RA), region B = [RA, NN)
WL = 24                # max window width, lo side
WH = 32                # max window width, hi side
SUB = 1024             # max idxs per gather call (HW-validated)
PADLN = 127.0          # localnode pad value (matches no iota column)
RDMA_AG = True         # replace CCE collectives with remote-DMA exchanges
NPIECE = 7             # shard exchange pieces (7 blocks each)
PB = NB // NPIECE      # blocks per piece


class P:
    """Compile-time structure: per-chunk slab windows (cross-core uniform)."""

    def __init__(self, wins_lo, wins_hi, NL):
        self.NL = NL
        self.nchunks = -(-NPAD // CH)
        # chunk k -> list of (slab_id, side, n0, w); slab ids number the
        # global slab stream: per chunk, lo slabs then hi slabs.
        self.chunks = []
        sid = 0
        for k in range(self.nchunks):
            c0 = k * CH
            lo = [(n0, w) for (n0, w) in wins_lo if n0 // CH == k]
            hi = [(n0, w) for (n0, w) in wins_hi if n0 // CH == k]
            ch = {"c0": c0, "W": min(CH, NPAD - c0), "lo": [], "hi": [],
                  "sl0_lo": sid}
            for n0, w in lo:
                ch["lo"].append((sid, n0, w))
                sid += 1
            ch["sl0_hi"] = sid
            for n0, w in hi:
                ch["hi"].append((sid, n0, w))
                sid += 1
            self.chunks.append(ch)
        self.NSLABS = sid
        self.SLOTS = sid * 128


def _wrap16(idx):
    n = len(idx)
    w = idx.reshape(n // 16, 16).T.astype(np.int16)
    return np.tile(w, (8, 1))


def _mk_windows(prof, wmax):
    wins = []
    p = 0
    while p < NPAD:
        chunk_end = min(p - p % CH + CH, NPAD)
        w = 0
        s = 0
        while p + w < chunk_end and w < wmax and s + prof[p + w] <= 128:
            s += prof[p + w]
            w += 1
        assert w > 0, f"degree > 128 at position {p}"
        if s > 0:
            wins.append((p, w))
        p += w
    return wins


def prep_inputs(x, edge_index, edge_weight, W1, b1, W2, b2, eps, gamma, beta, NL):
    src = np.asarray(edge_index[0], np.int64)
    dst = np.asarray(edge_index[1], np.int64)
    ew = np.asarray(edge_weight, np.float32)
    owner = src // NLOC

    cores = []
    lo_prof = np.zeros((C, NPAD), np.int64)
    hi_prof = np.zeros((C, NPAD), np.int64)
    for c in range(C):
        m = (dst // NLOC) == c
        sc, dc, wc = src[m], dst[m] - c * NLOC, ew[m]
        # section of each source in THIS core's table layout (XOR-relative:
        # Q7 resolves remote-dma relative dests by XOR of tpb ids)
        sec = (owner[m] ^ c) if RDMA_AG else owner[m]
        he = sec >= 5
        lo = np.bincount(dc[~he], minlength=NLOC)
        hi = np.bincount(dc[he], minlength=NLOC)
        keyhi = np.where(lo % 2 == 0, hi, 10**6 - hi)
        order = np.lexsort((keyhi, -lo))
        rank = np.empty(NLOC, np.int64)
        rank[order] = np.arange(NLOC)
        lo_prof[c, :NLOC] = lo[order]
        hi_prof[c, :NLOC] = hi[order]
        cores.append(dict(sc=sc, dc=dc, wc=wc, sec=sec, he=he,
                          order=order, rank=rank))

    # rank of every original node id within its owner's sorted shard
    rank_g = np.empty(NN, np.int64)
    for c in range(C):
        rank_g[c * NLOC : (c + 1) * NLOC] = cores[c]["rank"]

    wins_lo = _mk_windows(lo_prof.max(0), WL)
    wins_hi = _mk_windows(hi_prof.max(0), WH)
    p = P(wins_lo, wins_hi, NL)

    # node-position -> (slab id, col, window start) per side
    slab_of = {}
    for side, wins in (("lo", wins_lo), ("hi", wins_hi)):
        s_arr = np.full(NPAD, -1, np.int64)
        c_arr = np.zeros(NPAD, np.int64)
        for ch in p.chunks:
            for sid, n0, w in ch[side]:
                s_arr[n0 : n0 + w] = sid
                c_arr[n0 : n0 + w] = np.arange(w)
        slab_of[side] = (s_arr, c_arr)

    in_maps = []
    for c in range(C):
        cd = cores[c]
        gi = np.zeros(p.SLOTS, np.int64)
        wv = np.zeros(p.SLOTS, np.float32)
        ln = np.full(p.SLOTS, PADLN, np.float32)
        for side in ("lo", "hi"):
            he = cd["he"]
            m = he if side == "hi" else ~he
            es = cd["sec"][m] * NLOC + rank_g[cd["sc"][m]]
            if side == "hi":
                es = es - RA
            er = cd["rank"][cd["dc"][m]]   # dst position of each edge
            wc = cd["wc"][m]
            # per-core per-side degree of each position
            degp = np.bincount(er, minlength=NPAD)
            # within-node occurrence index
            o = np.argsort(er, kind="stable")
            inv = np.empty_like(o)
            inv[o] = np.arange(len(o))
            sorted_r = er[o]
            gstart = np.concatenate([[0], np.nonzero(np.diff(sorted_r))[0] + 1])
            start_of = np.zeros(len(o), np.int64)
            start_of[gstart] = gstart
            start_of = np.maximum.accumulate(start_of)
            occ = (np.arange(len(o)) - start_of)[inv]
            # node base offset inside its slab = sum of this core's degrees of
            # earlier positions in the window
            s_arr, c_arr = slab_of[side]
            csum = np.concatenate([[0], np.cumsum(degp)])
            win_n0 = np.zeros(NPAD, np.int64)
            for ch in p.chunks:
                for sid, n0, w in ch[side]:
                    win_n0[n0 : n0 + w] = n0
            base = csum[er] - csum[win_n0[er]]
            slot = s_arr[er] * 128 + base + occ
            assert s_arr[er].min() >= 0
            assert (base + occ).max() < 128
            gi[slot] = es
            wv[slot] = wc
            ln[slot] = c_arr[er]
        xs = np.asarray(x, np.float32)[c * NLOC : (c + 1) * NLOC][cd["order"]]
        pid = np.full(NPAD, -1, np.int64)
        pid[:NLOC] = cd["order"]
        in_maps.append({
            "gidx": _wrap16(gi),
            "pidx": _wrap16(pid),
            "lntab": np.ascontiguousarray(
                ln.reshape(p.NSLABS, 128).T.astype(BFNP)),
            "wtab": np.ascontiguousarray(
                wv.reshape(p.NSLABS, 128).T.astype(BFNP)),
            "x_own": xs.astype(BFNP),
        })

    # gather table for layer 0: per-core (rotated) sorted layout
    xf32 = np.asarray(x, np.float32)
    shard = [xf32[c * NLOC : (c + 1) * NLOC][cores[c]["order"]] for c in range(C)]
    for c in range(C):
        if RDMA_AG:
            xb = np.concatenate([shard[c ^ j] for j in range(C)])
        else:
            xb = np.concatenate(shard)
        in_maps[c]["xb"] = xb.astype(BFNP)
    epsv = np.asarray(eps, np.float32)
    identeps = np.zeros((D, NL * D), np.float32)
    for l in range(NL):
        identeps[:, l * D : (l + 1) * D] = np.eye(D) * (1.0 + epsv[l])
    vecs = np.zeros((D, 4 * NL), np.float32)
    vecs[:, 0 * NL : 1 * NL] = np.asarray(b1, np.float32).T
    vecs[:, 1 * NL : 2 * NL] = np.asarray(b2, np.float32).T
    vecs[:, 2 * NL : 3 * NL] = np.asarray(gamma, np.float32).T
    vecs[:, 3 * NL : 4 * NL] = np.asarray(beta, np.float32).T
    msl_lo = max(len(ch["lo"]) for ch in p.chunks)
    msl_hi = max(len(ch["hi"]) for ch in p.chunks)
    iota_lo = np.tile(np.tile(np.arange(WL, dtype=np.float32), msl_lo),
                      (128, 1)).astype(BFNP)
    iota_hi = np.tile(np.tile(np.arange(WH, dtype=np.float32), msl_hi),
                      (128, 1)).astype(BFNP)
    for im in in_maps:
        im.update({
            "w1": np.asarray(W1, np.float32).astype(BFNP),
            "w2": np.asarray(W2, np.float32).astype(BFNP),
            "vecs": vecs,
            "identeps": identeps.astype(BFNP),
            "identb": np.eye(D, dtype=np.float32).astype(BFNP),
            "iota_lo": iota_lo,
            "iota_hi": iota_hi,
        })
    return p, in_maps


def build_nc(p: P):
    nc = bacc.Bacc("TRN2", target_bir_lowering=False, debug=False, num_devices=C)
    NL = p.NL

    xb_ext = nc.dram_tensor("xb", [NN, D], BF16, kind="ExternalInput")
    x_own = nc.dram_tensor("x_own", [NLOC, D], BF16, kind="ExternalInput")
    gidx_ext = nc.dram_tensor("gidx", [128, p.SLOTS // 16], I16, kind="ExternalInput")
    pidx_ext = nc.dram_tensor("pidx", [128, NPAD // 16], I16, kind="ExternalInput")
    ln_ext = nc.dram_tensor("lntab", [128, p.NSLABS], BF16, kind="ExternalInput")
    wt_ext = nc.dram_tensor("wtab", [128, p.NSLABS], BF16, kind="ExternalInput")
    w1_ext = nc.dram_tensor("w1", [NL, D, D], BF16, kind="ExternalInput")
    w2_ext = nc.dram_tensor("w2", [NL, D, D], BF16, kind="ExternalInput")
    vecs_ext = nc.dram_tensor("vecs", [D, 4 * NL], F32, kind="ExternalInput")
    ideps_ext = nc.dram_tensor("identeps", [D, NL * D], BF16, kind="ExternalInput")
    identb_ext = nc.dram_tensor("identb", [D, D], BF16, kind="ExternalInput")
    msl_lo = max(len(ch["lo"]) for ch in p.chunks)
    msl_hi = max(len(ch["hi"]) for ch in p.chunks)
    iol_ext = nc.dram_tensor("iota_lo", [128, msl_lo * WL], BF16,
                             kind="ExternalInput")
    ioh_ext = nc.dram_tensor("iota_hi", [128, msl_hi * WH], BF16,
                             kind="ExternalInput")
    y_ext = nc.dram_tensor("y", [NLOC, D], F32, kind="ExternalOutput")

    xf_space = "Local" if RDMA_AG else "Shared"
    xf = [nc.dram_tensor(f"xfull{i}", [NN, D], BF16, kind="Internal",
                         addr_space=xf_space) for i in range(max(NL - 1, 1))]
    ccx = nc.dram_tensor("ccx", [NLOC, D], BF16, kind="Internal")
    st_in = nc.dram_tensor("st_in", [D, 2], F32, kind="Internal")
    st_out = [nc.dram_tensor(f"st_out{i}", [D, 2], F32, kind="Internal",
                             addr_space="Shared") for i in range(NL)]

    if RDMA_AG:
        sem_rx = nc.alloc_semaphore("ag_rx")
        sem_tx = nc.alloc_semaphore("ag_tx")
        sem_ack = nc.alloc_semaphore("ag_ack")
        sem_cp = nc.alloc_semaphore("ag_cp")
        sem_srx = nc.alloc_semaphore("st_rx")
        cnt = {"rx": 0, "tx": 0, "ack": 0, "cp": 0, "srx": 0, "Q": 0}

    with tile.TileContext(nc) as tc:
        nc.gpsimd.load_library(library_config.mlp)
        if RDMA_AG:
            # Clear exchange sems (sems persist across NEFF executions).
            # No barrier needed: a peer's earliest remote send happens a full
            # layer (~300us) after launch, far beyond any launch skew.
            for s in (sem_rx, sem_tx, sem_ack, sem_cp, sem_srx):
                nc.gpsimd.sem_clear(s)
        with (
            tc.tile_pool(name="const", bufs=1) as cpool,
            tc.tile_pool(name="big", bufs=1) as bpool,
            tc.tile_pool(name="msgp", bufs=2) as msgp,
            tc.tile_pool(name="apool", bufs=2) as apool,
            tc.tile_pool(name="grp", bufs=2) as grp,
            tc.tile_pool(name="tiny", bufs=1) as tiny,
            tc.tile_pool(name="ps_ag", bufs=2, space="PSUM") as ps_ag,
            tc.tile_pool(name="ps_mm", bufs=2, space="PSUM") as ps_mm,
            tc.tile_pool(name="ps_tp", bufs=2, space="PSUM") as ps_tp,
        ):
            # ---- constants ----
            w1s = cpool.tile([D, NL * D], BF16, tag="w1s")
            w2s = cpool.tile([D, NL * D], BF16, tag="w2s")
            vecs = cpool.tile([D, 4 * NL], F32, tag="vecs")
            ideps = cpool.tile([D, NL * D], BF16, tag="ideps")
            identb = cpool.tile([D, D], BF16, tag="identb")
            iol = cpool.tile([128, msl_lo * WL], BF16, tag="iol")
            ioh = cpool.tile([128, msl_hi * WH], BF16, tag="ioh")
            gidx = cpool.tile([128, p.SLOTS // 16], I16, tag="gidx")
            pidx = cpool.tile([128, NPAD // 16], I16, tag="pidx")
            lntab = cpool.tile([128, p.NSLABS], BF16, tag="lntab")
            wtab = cpool.tile([128, p.NSLABS], BF16, tag="wtab")
            if RDMA_AG:
                # remote-write landing zones (written by peers, invisible to
                # the tile dep tracker — guarded by manual semaphores)
                sstage = cpool.tile([128, 2, C - 1, PB, D], BF16, tag="sstage")
                sstat = cpool.tile([128, 2, 2 * (C - 1)], F32, tag="sstat")
            for l in range(NL):
                nc.sync.dma_start(out=w1s[:, l * D : (l + 1) * D], in_=w1_ext[l, :, :])
                nc.sync.dma_start(out=w2s[:, l * D : (l + 1) * D], in_=w2_ext[l, :, :])
            nc.sync.dma_start(out=vecs[:], in_=vecs_ext[:, :])
            nc.sync.dma_start(out=ideps[:], in_=ideps_ext[:, :])
            nc.sync.dma_start(out=identb[:], in_=identb_ext[:, :])
            nc.sync.dma_start(out=iol[:], in_=iol_ext[:, :])
            nc.sync.dma_start(out=ioh[:], in_=ioh_ext[:, :])
            nc.sync.dma_start(out=gidx[:], in_=gidx_ext[:, :])
            nc.sync.dma_start(out=pidx[:], in_=pidx_ext[:, :])
            nc.sync.dma_start(out=lntab[:], in_=ln_ext[:, :])
            nc.sync.dma_start(out=wtab[:], in_=wt_ext[:, :])

            def vcol(j, l):
                return vecs[:, j * NL + l : j * NL + l + 1]

            # ---- persistent node-major shard (sorted order), bf16 ----
            x_nm = bpool.tile([128, NB, D], BF16, tag="x_nm")
            h2_fm = bpool.tile([128, NPAD], BF16, tag="h2_fm")
            nc.vector.memset(x_nm[:, :, :], 0.0)
            NFB = NLOC // 128
            NRE = NLOC - NFB * 128
            if NFB:
                nc.sync.dma_start(
                    out=x_nm[:, 0:NFB, :],
                    in_=x_own.ap()[0 : NFB * 128, :].rearrange(
                        "(a p) d -> p a d", p=128),
                )
            if NRE:
                nc.sync.dma_start(
                    out=x_nm[0:NRE, NFB : NFB + 1, :],
                    in_=x_own.ap()[NFB * 128 : NLOC, :].rearrange(
                        "(a p) d -> p a d", p=NRE),
                )

            for l in range(NL):
                gsrc = xb_ext if l == 0 else xf[l - 1]
                viewA = gsrc.ap()[0:RA, :]
                viewB = gsrc.ap()[RA:NN, :]
                ssum = tiny.tile([128, p.nchunks], F32, tag="ssum")
                ssq = tiny.tile([128, p.nchunks], F32, tag="ssq")
                nc.vector.memset(ssum[:, :], 0.0)
                nc.vector.memset(ssq[:, :], 0.0)

                for k, ch in enumerate(p.chunks):
                    c0, W = ch["c0"], ch["W"]
                    nblk = W // 128
                    nsl_lo = len(ch["lo"])
                    nsl_hi = len(ch["hi"])

                    # ---- gathers (edge-major slabs), both regions ----
                    def gather(view, sl0, nsl, tag):
                        if nsl == 0:
                            return None
                        msg = msgp.tile([128, nsl, D], BF16, tag=tag)
                        s = 0
                        while s < nsl:
                            kk = min(SUB // 128, nsl - s)
                            nc.gpsimd.dma_gather(
                                out_ap=msg[:, s : s + kk, :],
                                in_ap=view,
                                idxs_ap=gidx[:, (sl0 + s) * 8 : (sl0 + s + kk) * 8],
                                num_idxs=kk * 128,
                                num_idxs_reg=kk * 128,
                                elem_size=D,
                            )
                            s += kk
                        return msg

                    msgl = gather(viewA, ch["sl0_lo"], nsl_lo, "msgl")
                    msgh = gather(viewB, ch["sl0_hi"], nsl_hi, "msgh")

                    # ---- A matrices: onehot(localnode) * w  ----
                    def build_A(sl0, nsl, wmax, iota, tag):
                        if nsl == 0:
                            return None
                        A = apool.tile([128, nsl, wmax], BF16, tag=tag)
                        ln_b = lntab[:, sl0 : sl0 + 1]
                        ln_b.ap = ln_b.ap[:1] + [[1, nsl], [0, wmax]]
                        io_b = iota[:, 0:1]
                        io_b.ap = io_b.ap[:1] + [[wmax, nsl], [1, wmax]]
                        nc.vector.tensor_tensor(
                            A[:, :, :], ln_b, io_b, Alu.is_equal)
                        wt_b = wtab[:, sl0 : sl0 + 1]
                        wt_b.ap = wt_b.ap[:1] + [[1, nsl], [0, wmax]]
                        nc.vector.tensor_mul(A[:, :, :], A[:, :, :], wt_b)
                        return A

                    Al = build_A(ch["sl0_lo"], nsl_lo, WL, iol, "Al")
                    Ah = build_A(ch["sl0_hi"], nsl_hi, WH, ioh, "Ah")

                    # ---- PSUM accumulate: (1+eps)x then weighted segment
                    # sums.  start=True only on the first matmul: it marks the
                    # whole 2KB zero-region pending-zero; all later matmuls
                    # accumulate (untouched bytes read as zero).
                    ps = ps_ag.tile([128, CH], F32, tag="ps")
                    for j in range(nblk):
                        b = c0 // 128 + j
                        nc.tensor.matmul(
                            ps[:, j * 128 : (j + 1) * 128],
                            x_nm[:, b, :], ideps[:, l * D : (l + 1) * D],
                            start=(j == 0), stop=False)
                    n_mm = nsl_lo + nsl_hi
                    mm = 0
                    for side, msg, A, wmax in (("lo", msgl, Al, WL),
                                               ("hi", msgh, Ah, WH)):
                        for i, (sid, n0, w) in enumerate(ch[side]):
                            mm += 1
                            nc.tensor.matmul(
                                ps[:, n0 - c0 : n0 - c0 + w],
                                msg[:, i, :],
                                A[:, i, 0:w],
                                start=False, stop=(mm == n_mm))

                    # ---- MLP ----
                    hfm = grp.tile([128, CH], BF16, tag="hfm")
                    nc.scalar.activation(out=hfm[:, 0:W], in_=ps[:, 0:W],
                                         func=Act.Copy)
                    ps1 = ps_mm.tile([128, CH], F32, tag="ps1")
                    nc.tensor.matmul(ps1[:, 0:W], w1s[:, l * D : (l + 1) * D],
                                     hfm[:, 0:W], start=True, stop=True)
                    g1 = grp.tile([128, CH], BF16, tag="g1")
                    nc.scalar.activation(out=g1[:, 0:W], in_=ps1[:, 0:W],
                                         func=Act.Relu, bias=vcol(0, l))
                    ps2 = ps_mm.tile([128, CH], F32, tag="ps2")
                    nc.tensor.matmul(ps2[:, 0:W], w2s[:, l * D : (l + 1) * D],
                                     g1[:, 0:W], start=True, stop=True)
                    wr = max(0, min(W, NLOC - c0))
                    if wr:
                        nc.vector.tensor_scalar(
                            out=h2_fm[:, c0 : c0 + wr], in0=ps2[:, 0:wr],
                            scalar1=vcol(1, l), scalar2=None,
                            op0=Alu.add, op1=Alu.add,
                            accum_out=ssum[:, k : k + 1])
                        sq = grp.tile([128, CH], BF16, tag="sq")
                        nc.scalar.activation(
                            out=sq[:, 0:wr], in_=h2_fm[:, c0 : c0 + wr],
                            func=Act.Square, accum_out=ssq[:, k : k + 1])
                    if W > wr:
                        nc.vector.tensor_scalar(
                            out=h2_fm[:, c0 + wr : c0 + W], in0=ps2[:, wr:W],
                            scalar1=vcol(1, l), scalar2=None, op0=Alu.add)

                # ---- BN stats: global sum across cores ----
                stl = tiny.tile([128, 2], F32, tag="stl")
                nc.vector.tensor_reduce(stl[:, 0:1], ssum[:, :], Ax.X, Alu.add)
                nc.vector.tensor_reduce(stl[:, 1:2], ssq[:, :], Ax.X, Alu.add)
                stg = tiny.tile([128, 2], F32, tag="stg")
                if RDMA_AG:
                    par = l % 2
                    t0 = tiny.tile([128, 2], F32, tag="t0")
                    with tc.tile_critical(name="stx"):
                        for j in range(1, C):
                            rd = [None] * C
                            rd[j] = (0, j)
                            nc.gpsimd.remote_dma_broadcast(
                                out_ap=sstat[:, par, 2 * (j - 1) : 2 * j],
                                in_ap=stl[:, :],
                                remote_sem=sem_srx, local_sem=sem_tx, rdests=rd)
                            cnt["tx"] += 16
                        nc.gpsimd.trigger_dma(count=None)
                        cnt["srx"] += 2 * (C - 1)
                        nc.vector.wait_ge(sem_srx, cnt["srx"])
                        ev = sstat[:, par, 0:1]
                        ev.ap = ev.ap[:1] + [[2, C - 1]]
                        nc.vector.tensor_reduce(t0[:, 0:1], ev, Ax.X, Alu.add)
                        od = sstat[:, par, 1:2]
                        od.ap = od.ap[:1] + [[2, C - 1]]
                        nc.vector.tensor_reduce(t0[:, 1:2], od, Ax.X, Alu.add)
                        nc.vector.tensor_add(stg[:, :], stl[:, :], t0[:, :])
                else:
                    nc.sync.dma_start(out=st_in[:, :], in_=stl[:, :])
                    nc.gpsimd.collective_compute(
                        "AllReduce", Alu.add, replica_groups=[list(range(C))],
                        ins=[st_in.ap().opt()], outs=[st_out[l].ap().opt()])
                    nc.sync.dma_start(out=stg[:, :], in_=st_out[l][:, :])
                mu = tiny.tile([128, 1], F32, tag="mu")
                var = tiny.tile([128, 1], F32, tag="var")
                rinv = tiny.tile([128, 1], F32, tag="rinv")
                scl = tiny.tile([128, 1], F32, tag="scl")
                sft = tiny.tile([128, 1], F32, tag="sft")
                tmp = tiny.tile([128, 1], F32, tag="tmp")
                nc.vector.tensor_scalar_mul(mu[:, :], stg[:, 0:1], 1.0 / NN)
                nc.vector.tensor_scalar_mul(var[:, :], stg[:, 1:2], 1.0 / NN)
                nc.vector.tensor_mul(tmp[:, :], mu[:, :], mu[:, :])
                nc.vector.tensor_sub(var[:, :], var[:, :], tmp[:, :])
                nc.vector.tensor_scalar_add(var[:, :], var[:, :], BN_EPS)
                nc.scalar.sqrt(var[:, :], var[:, :])
                nc.vector.reciprocal(rinv[:, :], var[:, :])
                nc.vector.tensor_mul(scl[:, :], rinv[:, :], vcol(2, l))
                nc.vector.tensor_mul(tmp[:, :], mu[:, :], scl[:, :])
                nc.vector.tensor_sub(sft[:, :], vcol(3, l), tmp[:, :])

                # ---- BN+ReLU, transpose back, residual ----
                for k, ch in enumerate(p.chunks):
                    c0, W = ch["c0"], ch["W"]
                    h3 = grp.tile([128, CH], BF16, tag="h3")
                    nc.scalar.activation(
                        out=h3[:, 0:W], in_=h2_fm[:, c0 : c0 + W],
                        func=Act.Relu, bias=sft[:, :], scale=scl[:, :])
                    for j in range(W // 128):
                        b = c0 // 128 + j
                        pt2 = ps_tp.tile([128, D], F32, tag="pt2")
                        nc.tensor.matmul(
                            pt2[:, :], h3[:, j * 128 : (j + 1) * 128],
                            identb[:, :], start=True, stop=True)
                        nc.vector.tensor_add(x_nm[:, b, :], x_nm[:, b, :],
                                             pt2[:, :])

                # ---- export ----
                if l < NL - 1 and RDMA_AG:
                    # push shard pieces into peers' SBUF staging; each core
                    # assembles its own (rotated) full table in local DRAM.
                    def copy_rows(sec, q, src_ap_full, src_ap_part):
                        # section-rows [q*PB*128, ...) -> xf[l] rows
                        r0 = sec * NLOC + q * PB * 128
                        nfull = PB
                        npart = 0
                        if q == NPIECE - 1:
                            nfull = (NLOC - q * PB * 128) // 128
                            npart = NLOC - q * PB * 128 - nfull * 128
                        if nfull:
                            nc.sync.dma_start(
                                out=xf[l].ap()[r0 : r0 + nfull * 128, :]
                                .rearrange("(a p) d -> p a d", p=128),
                                in_=src_ap_full(nfull)).then_inc(sem_cp, 16)
                            cnt["cp"] += 16
                        if npart:
                            nc.sync.dma_start(
                                out=xf[l].ap()[r0 + nfull * 128 :
                                               r0 + nfull * 128 + npart, :]
                                .rearrange("(a p) d -> p a d", p=npart),
                                in_=src_ap_part(nfull, npart)).then_inc(
                                    sem_cp, 16)
                            cnt["cp"] += 16

                    with tc.tile_critical(name="agx"):
                        for q in range(NPIECE):
                            Q = cnt["Q"]
                            cnt["Q"] += 1
                            if Q >= 2:
                                nc.gpsimd.wait_ge(sem_ack, 14 * (Q - 1))
                            b0 = q * PB
                            for j in range(1, C):
                                rd = [None] * C
                                rd[j] = (0, j)
                                nc.gpsimd.remote_dma_broadcast(
                                    out_ap=sstage[:, Q % 2, j - 1, :, :],
                                    in_ap=x_nm[:, b0 : b0 + PB, :],
                                    remote_sem=sem_rx, local_sem=sem_tx,
                                    rdests=rd)
                                cnt["tx"] += 16
                            nc.gpsimd.trigger_dma(count=None)
                            cnt["rx"] += 2 * (C - 1)
                            nc.sync.wait_ge(sem_rx, cnt["rx"])
                            # own shard piece -> section 0
                            copy_rows(
                                0, q,
                                lambda nf: x_nm[:, b0 : b0 + nf, :],
                                lambda nf, np_: x_nm[0:np_,
                                                     b0 + nf : b0 + nf + 1, :])
                            for j in range(1, C):
                                # slot j-1 holds the shard of core (me XOR j)
                                # = table section j
                                copy_rows(
                                    j, q,
                                    lambda nf, _j=j, _Q=Q: sstage[
                                        :, _Q % 2, _j - 1, 0:nf, :],
                                    lambda nf, np_, _j=j, _Q=Q: sstage[
                                        0:np_, _Q % 2, _j - 1, nf : nf + 1, :])
                            nc.gpsimd.wait_ge(sem_cp, cnt["cp"])
                            rd = [None] + [(0, j) for j in range(1, C)]
                            nc.gpsimd.remote_sem_update_broadcast(
                                remote_sem=sem_ack, local_sem=sem_tx, rdests=rd)
                            cnt["tx"] += 16
                            nc.gpsimd.trigger_dma(count=None)
                        # peers acked every piece => all my sends delivered,
                        # x_nm is safe to rewrite next layer
                        nc.gpsimd.wait_ge(sem_ack, 14 * cnt["Q"])
                elif l < NL - 1:
                    if NFB:
                        nc.sync.dma_start(
                            out=ccx.ap()[0 : NFB * 128, :].rearrange(
                                "(a p) d -> p a d", p=128),
                            in_=x_nm[:, 0:NFB, :])
                    if NRE:
                        nc.sync.dma_start(
                            out=ccx.ap()[NFB * 128 : NLOC, :].rearrange(
                                "(a p) d -> p a d", p=NRE),
                            in_=x_nm[0:NRE, NFB : NFB + 1, :])
                    nc.gpsimd.collective_compute(
                        "AllGather", Alu.bypass, replica_groups=[list(range(C))],
                        ins=[ccx.ap().opt()], outs=[xf[l].ap().opt()])
                else:
                    # final: zero y, convert shard to f32, dup-free scatter to
                    # original order
                    zt = grp.tile([128, CH], F32, tag="zt")
                    nc.vector.memset(zt[:, :], 0.0)
                    done = 0
                    while done < NLOC:
                        j = min(4, (NLOC - done) // 128)
                        if j > 0:
                            zv = zt[:, 0 : j * 128].rearrange(
                                "p (a d) -> p a d", a=j)
                            nc.sync.dma_start(
                                out=y_ext.ap()[done : done + j * 128, :]
                                .rearrange("(a p) d -> p a d", p=128),
                                in_=zv)
                            done += j * 128
                        else:
                            r = NLOC - done
                            nc.sync.dma_start(
                                out=y_ext.ap()[done:NLOC, :].rearrange(
                                    "(a p) d -> p a d", p=r),
                                in_=zt[0:r, 0:D].rearrange(
                                    "p (a d) -> p a d", a=1))
                            done = NLOC
                    yst = msgp.tile([128, NB, D], F32, tag="yst")
                    nc.scalar.activation(out=yst[:, :, :], in_=x_nm[:, :, :],
                                         func=Act.Copy)
                    s0 = 0
                    while s0 < NPAD:
                        n = min(1024, NPAD - s0)
                        nvalid = max(0, min(NLOC, s0 + n) - s0)
                        nc.gpsimd.dma_scatter_add(
                            out_ap=y_ext[:, :],
                            in_ap=yst[:, s0 // 128 : (s0 + n) // 128, :],
                            idxs_ap=pidx[:, s0 // 16 : (s0 + n) // 16],
                            num_idxs=n,
                            num_idxs_reg=nvalid,
                            elem_size=D,
                        )
                        s0 += n
    return nc


def run(inputs, NL=3, trace=False):
    p, in_maps = prep_inputs(
        inputs["x"], inputs["edge_index"], inputs["edge_weight"],
        inputs["W1"], inputs["b1"], inputs["W2"], inputs["b2"],
        inputs["eps"], inputs["gamma"], inputs["beta"], NL,
    )
    nc = build_nc(p)
    nc.compile()
    res = run_bass_kernel_spmd(nc, in_maps, core_ids=list(range(C)), trace=trace)
    y = np.concatenate([res.results[c]["y"] for c in range(C)], axis=0)
    return y, res


def kernel(**inputs):
    y, _ = run(inputs, NL=3)
    return y.astype(np.float32)
